# revision 16
# baseline (speedup 1.0000x reference)
"""Conv2d 3x3 (stride 1, pad 1) as implicit GEMM on 8 Trainium2 NeuronCores.

x: [32, 128, 56, 56] f32, W: [256, 128, 3, 3] f32 -> out: [32, 256, 56, 56] f32

Sharding: data-parallel over batch, 4 images per core.

Per-core kernel:
  - host pre-pads x to [4, 128, 58, 58] and casts to bf16
  - host pre-transposes W to [Cin=128, 9*Cout] bf16 (k-position major)
  - contraction dim Cin=128 lives on SBUF partitions; for each output tile
    (img, 8-row group, cout half) accumulate 9 matmuls (one per kernel tap)
    into one PSUM bank using strided views of the padded input
  - PSUM -> SBUF copy (fp32) -> streaming DMA store per tile
"""

import sys

for _p in ("/opt/trn_rl_repo",):
    if _p not in sys.path:
        sys.path.insert(0, _p)

import numpy as np
import ml_dtypes

import concourse.bass as bass
import concourse.bacc as bacc
import concourse.mybir as mybir
from concourse import tile
from concourse.bass_utils import run_bass_kernel_spmd

N_CORES = 8
B = 32
B_PER_CORE = B // N_CORES  # 4
CIN = 128
COUT = 256
H = W_DIM = 56
HP = WP = 58  # padded
KH = KW = 3
KPOS = KH * KW  # 9
ROWS = 8               # output rows per matmul
NG = H // ROWS         # 7 row groups
NFREE = ROWS * W_DIM   # 448 free dim per matmul (<= 512 psum bank)
COUT_TILES = COUT // 128  # 2

_NC_CACHE = None


def build_nc(reps: int = 1) -> bass.Bass:
    # Bacc (not raw Bass): its compile() runs move_matmul_waits_to_ldweights
    # and generate_event_semaphores, which split multi-wait instructions to
    # satisfy the 1-sync-wait-per-instruction hardware encoding limit.
    # reps > 1 repeats the compute+store body (same outputs) for slope-based
    # hardware timing; the shipped kernel uses reps=1.
    nc = bacc.Bacc()
    xp = nc.dram_tensor(
        "xp", [B_PER_CORE, CIN, HP * WP], mybir.dt.bfloat16, kind="ExternalInput"
    )
    wt = nc.dram_tensor(
        "wt", [CIN, KPOS * COUT], mybir.dt.bfloat16, kind="ExternalInput"
    )
    out = nc.dram_tensor(
        "out", [B_PER_CORE, COUT, H * W_DIM], mybir.dt.float32, kind="ExternalOutput"
    )

    with tile.TileContext(nc) as tc:
        with (
            tc.tile_pool(name="wpool", bufs=1) as wpool,
            tc.tile_pool(name="xpool", bufs=1) as xpool,
            tc.tile_pool(name="opool", bufs=6) as opool,
            tc.tile_pool(name="pspool", bufs=8, space="PSUM") as pspool,
        ):
            # Loads ride both HWDGE rings in parallel: weights (2 chunks) on
            # the scalar ring, x images (3 row chunks each) on the sync ring.
            # Chunking lets the first matmuls start as soon as weight chunk 0
            # and rows 0..17 of image 0 have landed; row chunk boundaries are
            # aligned so row group g only reads padded rows [8g, 8g+9].
            w_sb = wpool.tile([CIN, KPOS * COUT], mybir.dt.bfloat16, name="w_sb")
            WHALF = (KPOS * COUT) // 2
            nc.scalar.dma_start(w_sb[:, :WHALF], wt[:, :WHALF])
            nc.scalar.dma_start(w_sb[:, WHALF:], wt[:, WHALF:])

            XSPLITS = (0, 18, 34, HP)
            x_views = []
            for b in range(B_PER_CORE):
                xb = xpool.tile(
                    [CIN, HP * WP], mybir.dt.bfloat16, name=f"x_sb{b}", tag=f"x{b}"
                )
                for lo, hi in zip(XSPLITS[:-1], XSPLITS[1:]):
                    nc.sync.dma_start(
                        xb[:, lo * WP : hi * WP], xp[b, :, lo * WP : hi * WP]
                    )
                x_views.append(xb.rearrange("p (h w) -> p h w", w=WP))

            for _rep in range(reps):
              for b in range(B_PER_CORE):
                for c in range(COUT_TILES):
                    for g in range(NG):
                        r0 = g * ROWS
                        ps = pspool.tile(
                            [128, NFREE], mybir.dt.float32, name="ps", tag="ps"
                        )
                        for k in range(KPOS):
                            kh, kw = divmod(k, KW)
                            rhs = x_views[b][:, r0 + kh : r0 + kh + ROWS, kw : kw + W_DIM]
                            lhsT = w_sb[:, k * COUT + c * 128 : k * COUT + (c + 1) * 128]
                            nc.tensor.matmul(
                                ps, lhsT, rhs, start=(k == 0), stop=(k == KPOS - 1)
                            )
                        ob = opool.tile(
                            [128, NFREE], mybir.dt.float32, name="ob", tag="ob"
                        )
                        nc.vector.tensor_copy(ob, ps)
                        nc.sync.dma_start(
                            out[
                                b,
                                c * 128 : (c + 1) * 128,
                                r0 * W_DIM : (r0 + ROWS) * W_DIM,
                            ],
                            ob,
                        )
    nc.compile()
    return nc


def _get_nc() -> bass.Bass:
    global _NC_CACHE
    if _NC_CACHE is None:
        _NC_CACHE = build_nc()
    return _NC_CACHE


def _prep_inputs(x: np.ndarray, W: np.ndarray):
    x = np.asarray(x, dtype=np.float32)
    W = np.asarray(W, dtype=np.float32)
    bf16 = ml_dtypes.bfloat16

    xp = np.zeros((B, CIN, HP, WP), dtype=bf16)
    xp[:, :, 1 : 1 + H, 1 : 1 + W_DIM] = x.astype(bf16)
    xp = xp.reshape(B, CIN, HP * WP)

    # Wt[ci, k*COUT + co] = W[co, ci, kh, kw], k = kh*3 + kw
    Wt = (
        W.transpose(2, 3, 1, 0)          # [kh, kw, ci, co]
        .reshape(KPOS, CIN, COUT)        # [k, ci, co]
        .transpose(1, 0, 2)              # [ci, k, co]
        .reshape(CIN, KPOS * COUT)
        .astype(bf16)
    )

    in_maps = []
    for c in range(N_CORES):
        in_maps.append(
            {
                "xp": np.ascontiguousarray(xp[c * B_PER_CORE : (c + 1) * B_PER_CORE]),
                "wt": Wt,
            }
        )
    return in_maps


def kernel_run(x: np.ndarray, W: np.ndarray, **spmd_kwargs):
    """Run the conv and return (output, BassKernelResults)."""
    in_maps = _prep_inputs(x, W)
    res = run_bass_kernel_spmd(
        _get_nc(), in_maps, core_ids=list(range(N_CORES)), **spmd_kwargs
    )
    out = np.concatenate(
        [
            np.asarray(res.results[c]["out"], dtype=np.float32).reshape(
                B_PER_CORE, COUT, H, W_DIM
            )
            for c in range(N_CORES)
        ],
        axis=0,
    )
    return out, res


def kernel(x: np.ndarray, W: np.ndarray) -> np.ndarray:
    out, _ = kernel_run(x, W)
    return out


# revision 18
# speedup vs baseline: 1.0468x; 1.0468x over previous
"""Conv2d 3x3 (stride 1, pad 1) as implicit GEMM on 8 Trainium2 NeuronCores.

x: [32, 128, 56, 56] f32, W: [256, 128, 3, 3] f32 -> out: [32, 256, 56, 56] f32

Sharding: data-parallel over batch, 4 images per core.

Per-core kernel:
  - host pre-pads x to [4, 128, 58, 58] and casts to bf16
  - host pre-transposes W to [Cin=128, 9*Cout] bf16 (k-position major)
  - contraction dim Cin=128 lives on SBUF partitions; for each output tile
    (img, 8-row group, cout half) accumulate 9 matmuls (one per kernel tap)
    into one PSUM bank using strided views of the padded input
  - PSUM -> SBUF copy (fp32) -> streaming DMA store per tile
"""

import sys

for _p in ("/opt/trn_rl_repo",):
    if _p not in sys.path:
        sys.path.insert(0, _p)

import numpy as np
import ml_dtypes

import concourse.bass as bass
import concourse.bacc as bacc
import concourse.mybir as mybir
from concourse import tile
from concourse.bass_utils import run_bass_kernel_spmd

N_CORES = 8
B = 32
B_PER_CORE = B // N_CORES  # 4
CIN = 128
COUT = 256
H = W_DIM = 56
HP = WP = 58  # padded
KH = KW = 3
KPOS = KH * KW  # 9
ROWS = 8               # output rows per matmul
NG = H // ROWS         # 7 row groups
NFREE = ROWS * W_DIM   # 448 free dim per matmul (<= 512 psum bank)
COUT_TILES = COUT // 128  # 2

_NC_CACHE = None


def build_nc(reps: int = 1) -> bass.Bass:
    # Bacc (not raw Bass): its compile() runs move_matmul_waits_to_ldweights
    # and generate_event_semaphores, which split multi-wait instructions to
    # satisfy the 1-sync-wait-per-instruction hardware encoding limit.
    # reps > 1 repeats the compute+store body (same outputs) for slope-based
    # hardware timing; the shipped kernel uses reps=1.
    nc = bacc.Bacc()
    xp = nc.dram_tensor(
        "xp", [B_PER_CORE, CIN, HP * WP], mybir.dt.bfloat16, kind="ExternalInput"
    )
    wt = nc.dram_tensor(
        "wt", [CIN, KPOS * COUT], mybir.dt.bfloat16, kind="ExternalInput"
    )
    out = nc.dram_tensor(
        "out", [B_PER_CORE, COUT, H * W_DIM], mybir.dt.float32, kind="ExternalOutput"
    )

    with tile.TileContext(nc) as tc:
        with (
            tc.tile_pool(name="wpool", bufs=1) as wpool,
            tc.tile_pool(name="xpool", bufs=1) as xpool,
            tc.tile_pool(name="opool", bufs=6) as opool,
            tc.tile_pool(name="pspool", bufs=7, space="PSUM") as pspool,
            tc.tile_pool(name="warmpool", bufs=1, space="PSUM") as warmpool,
        ):
            # Warm the PE clock (HAM / p-state ramp) while the input DMAs are
            # in flight: a chain of dependency-free matmuls on a memset
            # scratch tile keeps the PE busy from t=0, so the real matmuls
            # start at full clock. These never block the real stream (they
            # are ahead of it in PE program order and wait on nothing).
            scratch = opool.tile([128, 64], mybir.dt.bfloat16, name="warm_src", tag="wsrc")
            nc.vector.memset(scratch, 0.0)
            warm_ps = warmpool.tile([64, 64], mybir.dt.float32, name="warm_ps", tag="wps")
            for _ in range(64):
                nc.tensor.matmul(warm_ps, scratch[:, :64], scratch, start=True, stop=True)
            # Loads ride both HWDGE rings in parallel: weights (2 chunks) on
            # the scalar ring, x images (3 row chunks each) on the sync ring.
            # Chunking lets the first matmuls start as soon as weight chunk 0
            # and rows 0..17 of image 0 have landed; row chunk boundaries are
            # aligned so row group g only reads padded rows [8g, 8g+9].
            w_sb = wpool.tile([CIN, KPOS * COUT], mybir.dt.bfloat16, name="w_sb")
            WSPLITS = (0, 3 * COUT, 6 * COUT, KPOS * COUT)
            for lo, hi in zip(WSPLITS[:-1], WSPLITS[1:]):
                nc.scalar.dma_start(w_sb[:, lo:hi], wt[:, lo:hi])

            XSPLITS = (0, 10, 18, 34, HP)
            x_views = []
            for b in range(B_PER_CORE):
                xb = xpool.tile(
                    [CIN, HP * WP], mybir.dt.bfloat16, name=f"x_sb{b}", tag=f"x{b}"
                )
                for lo, hi in zip(XSPLITS[:-1], XSPLITS[1:]):
                    nc.sync.dma_start(
                        xb[:, lo * WP : hi * WP], xp[b, :, lo * WP : hi * WP]
                    )
                x_views.append(xb.rearrange("p (h w) -> p h w", w=WP))

            for _rep in range(reps):
              for b in range(B_PER_CORE):
                for c in range(COUT_TILES):
                    for g in range(NG):
                        r0 = g * ROWS
                        ps = pspool.tile(
                            [128, NFREE], mybir.dt.float32, name="ps", tag="ps"
                        )
                        for k in range(KPOS):
                            kh, kw = divmod(k, KW)
                            rhs = x_views[b][:, r0 + kh : r0 + kh + ROWS, kw : kw + W_DIM]
                            lhsT = w_sb[:, k * COUT + c * 128 : k * COUT + (c + 1) * 128]
                            nc.tensor.matmul(
                                ps, lhsT, rhs, start=(k == 0), stop=(k == KPOS - 1)
                            )
                        ob = opool.tile(
                            [128, NFREE], mybir.dt.float32, name="ob", tag="ob"
                        )
                        nc.vector.tensor_copy(ob, ps)
                        nc.sync.dma_start(
                            out[
                                b,
                                c * 128 : (c + 1) * 128,
                                r0 * W_DIM : (r0 + ROWS) * W_DIM,
                            ],
                            ob,
                        )
    nc.compile()
    return nc


def _get_nc() -> bass.Bass:
    global _NC_CACHE
    if _NC_CACHE is None:
        _NC_CACHE = build_nc()
    return _NC_CACHE


def _prep_inputs(x: np.ndarray, W: np.ndarray):
    x = np.asarray(x, dtype=np.float32)
    W = np.asarray(W, dtype=np.float32)
    bf16 = ml_dtypes.bfloat16

    xp = np.zeros((B, CIN, HP, WP), dtype=bf16)
    xp[:, :, 1 : 1 + H, 1 : 1 + W_DIM] = x.astype(bf16)
    xp = xp.reshape(B, CIN, HP * WP)

    # Wt[ci, k*COUT + co] = W[co, ci, kh, kw], k = kh*3 + kw
    Wt = (
        W.transpose(2, 3, 1, 0)          # [kh, kw, ci, co]
        .reshape(KPOS, CIN, COUT)        # [k, ci, co]
        .transpose(1, 0, 2)              # [ci, k, co]
        .reshape(CIN, KPOS * COUT)
        .astype(bf16)
    )

    in_maps = []
    for c in range(N_CORES):
        in_maps.append(
            {
                "xp": np.ascontiguousarray(xp[c * B_PER_CORE : (c + 1) * B_PER_CORE]),
                "wt": Wt,
            }
        )
    return in_maps


def kernel_run(x: np.ndarray, W: np.ndarray, **spmd_kwargs):
    """Run the conv and return (output, BassKernelResults)."""
    in_maps = _prep_inputs(x, W)
    res = run_bass_kernel_spmd(
        _get_nc(), in_maps, core_ids=list(range(N_CORES)), **spmd_kwargs
    )
    out = np.concatenate(
        [
            np.asarray(res.results[c]["out"], dtype=np.float32).reshape(
                B_PER_CORE, COUT, H, W_DIM
            )
            for c in range(N_CORES)
        ],
        axis=0,
    )
    return out, res


def kernel(x: np.ndarray, W: np.ndarray) -> np.ndarray:
    out, _ = kernel_run(x, W)
    return out


# revision 23
# speedup vs baseline: 1.0559x; 1.0087x over previous
"""Conv2d 3x3 (stride 1, pad 1) as implicit GEMM on 8 Trainium2 NeuronCores.

x: [32, 128, 56, 56] f32, W: [256, 128, 3, 3] f32 -> out: [32, 256, 56, 56] f32

Sharding: data-parallel over batch, 4 images per core.

Per-core kernel:
  - host pre-pads x to [4, 128, 58, 58] and casts to bf16
  - host pre-transposes W to [Cin=128, 9*Cout] bf16 (k-position major)
  - contraction dim Cin=128 lives on SBUF partitions; for each output tile
    (img, 8-row group, cout half) accumulate 9 matmuls (one per kernel tap)
    into one PSUM bank using strided views of the padded input
  - PSUM -> SBUF copy (fp32) -> streaming DMA store per tile
"""

import sys

for _p in ("/opt/trn_rl_repo",):
    if _p not in sys.path:
        sys.path.insert(0, _p)

import numpy as np
import ml_dtypes

import concourse.bass as bass
import concourse.bacc as bacc
import concourse.mybir as mybir
from concourse import tile
from concourse.bass_utils import run_bass_kernel_spmd

N_CORES = 8
B = 32
B_PER_CORE = B // N_CORES  # 4
CIN = 128
COUT = 256
H = W_DIM = 56
HP = WP = 58  # padded
KH = KW = 3
KPOS = KH * KW  # 9
ROWS = 8               # output rows per matmul
NG = H // ROWS         # 7 row groups
NFREE = ROWS * W_DIM   # 448 free dim per matmul (<= 512 psum bank)
COUT_TILES = COUT // 128  # 2

_NC_CACHE = None


def build_nc(reps: int = 1, xsplits=(0, 10, 18, 34, HP), wchunks: int = 2) -> bass.Bass:
    # Bacc (not raw Bass): its compile() runs move_matmul_waits_to_ldweights
    # and generate_event_semaphores, which split multi-wait instructions to
    # satisfy the 1-sync-wait-per-instruction hardware encoding limit.
    # reps > 1 repeats the compute+store body (same outputs) for slope-based
    # hardware timing; the shipped kernel uses reps=1.
    nc = bacc.Bacc()
    xp = nc.dram_tensor(
        "xp", [B_PER_CORE, CIN, HP * WP], mybir.dt.bfloat16, kind="ExternalInput"
    )
    wt = nc.dram_tensor(
        "wt", [CIN, KPOS * COUT], mybir.dt.bfloat16, kind="ExternalInput"
    )
    out = nc.dram_tensor(
        "out", [B_PER_CORE, COUT, H * W_DIM], mybir.dt.float32, kind="ExternalOutput"
    )

    with tile.TileContext(nc) as tc:
        with (
            tc.tile_pool(name="wpool", bufs=1) as wpool,
            tc.tile_pool(name="xpool", bufs=1) as xpool,
            tc.tile_pool(name="opool", bufs=6) as opool,
            tc.tile_pool(name="pspool", bufs=7, space="PSUM") as pspool,
            tc.tile_pool(name="warmpool", bufs=1, space="PSUM") as warmpool,
        ):
            # Warm the PE clock (HAM / p-state ramp) while the input DMAs are
            # in flight: a chain of dependency-free matmuls on a memset
            # scratch tile keeps the PE busy from t=0, so the real matmuls
            # start at full clock. These never block the real stream (they
            # are ahead of it in PE program order and wait on nothing).
            scratch = opool.tile([128, 64], mybir.dt.bfloat16, name="warm_src", tag="wsrc")
            nc.vector.memset(scratch, 0.0)
            warm_ps = warmpool.tile([64, 64], mybir.dt.float32, name="warm_ps", tag="wps")
            for _ in range(64):
                nc.tensor.matmul(warm_ps, scratch[:, :64], scratch, start=True, stop=True)
            # Loads ride both HWDGE rings in parallel: weights (2 chunks) on
            # the scalar ring, x images (3 row chunks each) on the sync ring.
            # Chunking lets the first matmuls start as soon as weight chunk 0
            # and rows 0..17 of image 0 have landed; row chunk boundaries are
            # aligned so row group g only reads padded rows [8g, 8g+9].
            w_sb = wpool.tile([CIN, KPOS * COUT], mybir.dt.bfloat16, name="w_sb")
            WSPLITS = tuple(
                (KPOS * COUT) * i // wchunks for i in range(wchunks)
            ) + (KPOS * COUT,)
            for lo, hi in zip(WSPLITS[:-1], WSPLITS[1:]):
                nc.scalar.dma_start(w_sb[:, lo:hi], wt[:, lo:hi])

            x_views = []
            for b in range(B_PER_CORE):
                xb = xpool.tile(
                    [CIN, HP * WP], mybir.dt.bfloat16, name=f"x_sb{b}", tag=f"x{b}"
                )
                # Only image 0 races the PE; later images load as one DMA.
                splits = tuple(xsplits) if b == 0 else (0, HP)
                for lo, hi in zip(splits[:-1], splits[1:]):
                    nc.sync.dma_start(
                        xb[:, lo * WP : hi * WP], xp[b, :, lo * WP : hi * WP]
                    )
                x_views.append(xb.rearrange("p (h w) -> p h w", w=WP))

            for _rep in range(reps):
              for b in range(B_PER_CORE):
                for g in range(NG):
                    for c in range(COUT_TILES):
                        r0 = g * ROWS
                        ps = pspool.tile(
                            [128, NFREE], mybir.dt.float32, name="ps", tag="ps"
                        )
                        for k in range(KPOS):
                            kh, kw = divmod(k, KW)
                            rhs = x_views[b][:, r0 + kh : r0 + kh + ROWS, kw : kw + W_DIM]
                            lhsT = w_sb[:, k * COUT + c * 128 : k * COUT + (c + 1) * 128]
                            nc.tensor.matmul(
                                ps, lhsT, rhs, start=(k == 0), stop=(k == KPOS - 1)
                            )
                        ob = opool.tile(
                            [128, NFREE], mybir.dt.float32, name="ob", tag="ob"
                        )
                        nc.vector.tensor_copy(ob, ps)
                        nc.sync.dma_start(
                            out[
                                b,
                                c * 128 : (c + 1) * 128,
                                r0 * W_DIM : (r0 + ROWS) * W_DIM,
                            ],
                            ob,
                        )
    nc.compile()
    return nc


def _get_nc() -> bass.Bass:
    global _NC_CACHE
    if _NC_CACHE is None:
        _NC_CACHE = build_nc()
    return _NC_CACHE


def _prep_inputs(x: np.ndarray, W: np.ndarray):
    x = np.asarray(x, dtype=np.float32)
    W = np.asarray(W, dtype=np.float32)
    bf16 = ml_dtypes.bfloat16

    xp = np.zeros((B, CIN, HP, WP), dtype=bf16)
    xp[:, :, 1 : 1 + H, 1 : 1 + W_DIM] = x.astype(bf16)
    xp = xp.reshape(B, CIN, HP * WP)

    # Wt[ci, k*COUT + co] = W[co, ci, kh, kw], k = kh*3 + kw
    Wt = (
        W.transpose(2, 3, 1, 0)          # [kh, kw, ci, co]
        .reshape(KPOS, CIN, COUT)        # [k, ci, co]
        .transpose(1, 0, 2)              # [ci, k, co]
        .reshape(CIN, KPOS * COUT)
        .astype(bf16)
    )

    in_maps = []
    for c in range(N_CORES):
        in_maps.append(
            {
                "xp": np.ascontiguousarray(xp[c * B_PER_CORE : (c + 1) * B_PER_CORE]),
                "wt": Wt,
            }
        )
    return in_maps


def kernel_run(x: np.ndarray, W: np.ndarray, **spmd_kwargs):
    """Run the conv and return (output, BassKernelResults)."""
    in_maps = _prep_inputs(x, W)
    res = run_bass_kernel_spmd(
        _get_nc(), in_maps, core_ids=list(range(N_CORES)), **spmd_kwargs
    )
    out = np.concatenate(
        [
            np.asarray(res.results[c]["out"], dtype=np.float32).reshape(
                B_PER_CORE, COUT, H, W_DIM
            )
            for c in range(N_CORES)
        ],
        axis=0,
    )
    return out, res


def kernel(x: np.ndarray, W: np.ndarray) -> np.ndarray:
    out, _ = kernel_run(x, W)
    return out
